# revision 12
# baseline (speedup 1.0000x reference)
"""Trainium2 Bass kernel for nn_ConvexGenerator (MoE-routed convex generator).

Expert-parallel sharding: core c owns class c's IGU weights (Wa[c], ba[c]) and
class buffer Xbuf[c]. Samples are routed by class_ids on the host (the
"all-to-all" is the host-side shard/unshard), so each core computes logits
only over its own class's columns -- the 8x headroom over the dense reference.

Per-core pipeline (all activations kept transposed, [feature, sample]):
  t  = gelu(gelu([z, onehot] @ W1 + b1) @ W2 + b2)         TensorE + ScalarE
  e  = exp(Wa_c.T @ t + ba_masked)  (bias folds mask+ba)   TensorE + ScalarE
  out= (e.T @ [X | 1]) ; num/den split off the ones column TensorE + VectorE
Softmax max-subtraction is skipped: logits are O(0.2) here, and masked columns
use a -1e9 bias so exp underflows to exactly 0.
"""

import os

import numpy as np

P = 128
B0 = 2048
LATENT = 128
C = 8
HID = 1024
D = 512
NMAX = 4096
COUNTS = np.array([1024, 1536, 2048, 2560, 3072, 3584, 3840, 4096])
NEG = -1e9
KC = HID // P     # 8 contraction chunks
NCH = NMAX // P   # 32 column chunks per class
DXT = D + 4       # X | ones | zero pad (even sizes for fp32r matmul)

_cache: dict = {}


def _build(cap: int):
    """Build + compile the per-core Tile program for sample capacity `cap`."""
    from contextlib import ExitStack

    import concourse.bacc as bacc
    import concourse.mybir as mybir
    import concourse.tile as tile

    f32 = mybir.dt.float32
    f32r = mybir.dt.float32r
    bf16 = mybir.dt.bfloat16
    f16 = mybir.dt.float16
    AF = mybir.ActivationFunctionType

    nc = bacc.Bacc("TRN2", target_bir_lowering=False, debug=False,
                   enable_asserts=False, num_devices=8)

    zT_d = nc.dram_tensor("zT", [P, cap], f32r, kind="ExternalInput")
    W1z_d = nc.dram_tensor("W1z", [P, HID], f32r, kind="ExternalInput")
    b1c_d = nc.dram_tensor("b1c", [P, KC], f32, kind="ExternalInput")
    W2r_d = nc.dram_tensor("W2r", [P, KC, HID], f32r, kind="ExternalInput")
    b2r_d = nc.dram_tensor("b2r", [P, KC], f32, kind="ExternalInput")
    Wap_d = nc.dram_tensor("Wap", [NCH, P, HID], f16, kind="ExternalInput")
    bac_d = nc.dram_tensor("bac", [P, NCH], f32, kind="ExternalInput")
    Xp_d = nc.dram_tensor("Xp", [NCH, P, DXT], f16, kind="ExternalInput")
    out_d = nc.dram_tensor("out", [cap, D], f32, kind="ExternalOutput")

    n_st = (cap + P - 1) // P          # 128-sample tiles for the combine
    SGS = 512                          # fp32 moving-operand free-dim limit
    sgroups = [(g, min(SGS, cap - g)) for g in range(0, cap, SGS)]

    with tile.TileContext(nc) as tc, ExitStack() as ctx:
        consts = ctx.enter_context(tc.tile_pool(name="consts", bufs=1))
        wa_pool = ctx.enter_context(tc.tile_pool(name="wa", bufs=3))
        psA = ctx.enter_context(tc.tile_pool(name="psA", bufs=3, space="PSUM"))
        psL = psA
        psC = ctx.enter_context(tc.tile_pool(name="psC", bufs=2, space="PSUM"))
        outp = ctx.enter_context(tc.tile_pool(name="outp", bufs=2))

        zT_sb = consts.tile([P, cap], f32r)
        nc.sync.dma_start(zT_sb[:], zT_d[:])
        W1z_sb = consts.tile([P, HID], f32r)
        nc.sync.dma_start(W1z_sb[:], W1z_d[:])
        b1c_sb = consts.tile([P, KC], f32)
        nc.sync.dma_start(b1c_sb[:], b1c_d[:])
        W2_sb = consts.tile([P, KC, HID], f32r)
        for k in range(KC):
            nc.sync.dma_start(W2_sb[:, k, :], W2r_d[:, k, :])
        b2r_sb = consts.tile([P, KC], f32)
        nc.sync.dma_start(b2r_sb[:], b2r_d[:])
        bac_sb = consts.tile([P, NCH], f32)
        nc.sync.dma_start(bac_sb[:], bac_d[:])
        x_all = consts.tile([P, NCH, DXT], f16)

        # ---- Phase A: cTMU (two gelu layers), activations as [hid, sample]
        h_sb = consts.tile([P, KC, cap], f32r)
        t_sb = consts.tile([P, KC, cap], f16)
        for (s0, slen) in sgroups:
            for j in range(KC):
                ph = psA.tile([P, slen], f32, tag="ps_mlp")
                nc.tensor.matmul(ph[:], W1z_sb[:, j * P:(j + 1) * P],
                                 zT_sb[:, s0:s0 + slen],
                                 start=True, stop=True)
                nc.scalar.activation(h_sb[:, j, s0:s0 + slen], ph[:], AF.Gelu,
                                     bias=b1c_sb[:, j:j + 1])
            for j in range(KC):
                pt = psA.tile([P, slen], f32, tag="ps_mlp")
                for k in range(KC):
                    nc.tensor.matmul(pt[:], W2_sb[:, k, j * P:(j + 1) * P],
                                     h_sb[:, k, s0:s0 + slen],
                                     start=(k == 0), stop=(k == KC - 1))
                nc.scalar.activation(t_sb[:, j, s0:s0 + slen], pt[:], AF.Gelu,
                                     bias=b2r_sb[:, j:j + 1])

        # ---- Phase B: routed IGU logits + fused mask/bias/exp
        e_all = consts.tile([P, NCH, cap], f16)
        for i in range(NCH):
            wa_t = wa_pool.tile([P, HID], f16, tag="wa")
            nc.sync.dma_start(wa_t[:], Wap_d[i])
            nc.sync.dma_start(x_all[:, i, :], Xp_d[i])
            for (s0, slen) in sgroups:
                pl = psL.tile([P, slen], f32, tag="ps_mlp")
                for k in range(KC):
                    nc.tensor.matmul(pl[:], wa_t[:, k * P:(k + 1) * P],
                                     t_sb[:, k, s0:s0 + slen],
                                     start=(k == 0), stop=(k == KC - 1))
                nc.scalar.activation(e_all[:, i, s0:s0 + slen], pl[:], AF.Exp,
                                     bias=bac_sb[:, i:i + 1])

        # ---- Phase C: convex combination; ones-column of Xp gives the denom
        for st in range(n_st):
            sz = min(P, cap - st * P)
            pa = psC.tile([P, 256], f32, tag="pa")
            pb = psC.tile([P, 258], f32, tag="pb")
            for i in range(NCH):
                lhs = e_all[:, i, st * P:st * P + sz]
                nc.tensor.matmul(pa[:sz, :], lhs,
                                 x_all[:, i, 0:256],
                                 start=(i == 0), stop=(i == NCH - 1))
                nc.tensor.matmul(pb[:sz, :], lhs,
                                 x_all[:, i, 256:514],
                                 start=(i == 0), stop=(i == NCH - 1))
            r = outp.tile([P, 1], f32, tag="recip")
            nc.vector.reciprocal(r[:sz], pb[:sz, 256:257])
            o = outp.tile([P, D], f32, tag="out")
            nc.vector.tensor_scalar_mul(o[:sz, 0:256], pa[:sz, :], r[:sz])
            nc.vector.tensor_scalar_mul(o[:sz, 256:512], pb[:sz, 0:256], r[:sz])
            nc.sync.dma_start(out_d[st * P:st * P + sz, :], o[:sz, :])

    nc.compile()
    return nc


def _get_compiled(cap: int):
    if cap not in _cache:
        _cache[cap] = _build(cap)
    return _cache[cap]


def kernel(z, class_ids, W1, b1, W2, b2, Wa, ba, Xbuf):
    from concourse.bass_utils import run_bass_kernel_spmd

    z = np.ascontiguousarray(np.asarray(z, np.float32))
    class_ids = np.asarray(class_ids).astype(np.int64)
    W1 = np.asarray(W1, np.float32)
    b1 = np.asarray(b1, np.float32)
    W2 = np.asarray(W2, np.float32)
    b2 = np.asarray(b2, np.float32)
    Wa = np.asarray(Wa, np.float32)
    ba = np.asarray(ba, np.float32)
    Xbuf = np.asarray(Xbuf, np.float32)

    B = z.shape[0]
    order = np.argsort(class_ids, kind="stable")
    counts = np.bincount(class_ids, minlength=C)
    cap = max(64, int(-(-counts.max() // 32) * 32))

    nc = _get_compiled(cap)

    W1z = np.ascontiguousarray(W1[:LATENT])
    W2r = np.ascontiguousarray(W2.reshape(KC, P, HID).transpose(1, 0, 2))
    b2r = np.ascontiguousarray(b2.reshape(KC, P).T)

    in_maps = []
    idx_by_class = []
    off = 0
    for c in range(C):
        n_c = int(counts[c])
        idx = order[off:off + n_c]
        off += n_c
        idx_by_class.append(idx)

        zTc = np.zeros((P, cap), np.float32)
        zTc[:, :n_c] = z[idx].T
        b1c = np.ascontiguousarray((b1 + W1[LATENT + c]).reshape(KC, P).T)
        Wap = np.ascontiguousarray(
            Wa[c].reshape(KC, P, NCH, P).transpose(2, 1, 0, 3).reshape(NCH, P, HID)
        ).astype(np.float16)
        bam = np.where(np.arange(NMAX) < COUNTS[c], ba[c], NEG).astype(np.float32)
        bac = np.ascontiguousarray(bam.reshape(NCH, P).T)
        Xp = np.zeros((NCH, P, DXT), np.float16)
        Xp[:, :, :D] = Xbuf[c].reshape(NCH, P, D)
        Xp[:, :, D] = 1.0

        in_maps.append({
            "zT": zTc, "W1z": W1z, "b1c": b1c, "W2r": W2r, "b2r": b2r,
            "Wap": Wap, "bac": bac, "Xp": np.ascontiguousarray(Xp),
        })

    trace = bool(os.environ.get("BASS_TRACE"))
    res = run_bass_kernel_spmd(
        nc, in_maps, core_ids=list(range(8)),
        trace=trace,
        trace_cores=list(range(8)) if trace else None,
    )
    global _last_results
    _last_results = res

    out = np.zeros((B, D), np.float32)
    for c in range(C):
        n_c = int(counts[c])
        if n_c:
            out[idx_by_class[c]] = res.results[c]["out"][:n_c]
    return out


_last_results = None


# revision 15
# speedup vs baseline: 1.0236x; 1.0236x over previous
"""Trainium2 Bass kernel for nn_ConvexGenerator (MoE-routed convex generator).

Expert-parallel sharding: core c owns class c's IGU weights (Wa[c], ba[c]) and
class buffer Xbuf[c]. Samples are routed by class_ids on the host (the
"all-to-all" is the host-side shard/unshard), so each core computes logits
only over its own class's columns -- the 8x headroom over the dense reference.

Per-core pipeline (all activations kept transposed, [feature, sample]):
  t  = gelu(gelu([z, onehot] @ W1 + b1) @ W2 + b2)         TensorE + ScalarE
  e  = exp(Wa_c.T @ t + ba_masked)  (bias folds mask+ba)   TensorE + ScalarE
  out= (e.T @ [X | 1]) ; num/den split off the ones column TensorE + VectorE
Softmax max-subtraction is skipped: logits are O(0.2) here, and masked columns
use a -1e9 bias so exp underflows to exactly 0.
"""

import os

import numpy as np

P = 128
B0 = 2048
LATENT = 128
C = 8
HID = 1024
D = 512
NMAX = 4096
COUNTS = np.array([1024, 1536, 2048, 2560, 3072, 3584, 3840, 4096])
NEG = -1e9
KC = HID // P     # 8 contraction chunks
NCH = NMAX // P   # 32 column chunks per class
DXT = D + 4       # X | ones | zero pad (even sizes for fp32r matmul)

_cache: dict = {}


def _build(cap: int):
    """Build + compile the per-core Tile program for sample capacity `cap`."""
    from contextlib import ExitStack

    import concourse.bacc as bacc
    import concourse.mybir as mybir
    import concourse.tile as tile

    f32 = mybir.dt.float32
    f32r = mybir.dt.float32r
    bf16 = mybir.dt.bfloat16
    f16 = mybir.dt.float16
    AF = mybir.ActivationFunctionType

    nc = bacc.Bacc("TRN2", target_bir_lowering=False, debug=False,
                   enable_asserts=False, num_devices=8)

    zT_d = nc.dram_tensor("zT", [P, cap], f32r, kind="ExternalInput")
    W1z_d = nc.dram_tensor("W1z", [P, HID], f32r, kind="ExternalInput")
    b1c_d = nc.dram_tensor("b1c", [P, KC], f32, kind="ExternalInput")
    W2r_d = nc.dram_tensor("W2r", [P, KC, HID], f32r, kind="ExternalInput")
    b2r_d = nc.dram_tensor("b2r", [P, KC], f32, kind="ExternalInput")
    Wap_d = nc.dram_tensor("Wap", [NCH, P, HID], f16, kind="ExternalInput")
    bac_d = nc.dram_tensor("bac", [P, NCH], f32, kind="ExternalInput")
    Xp_d = nc.dram_tensor("Xp", [NCH, P, DXT], f16, kind="ExternalInput")
    out_d = nc.dram_tensor("out", [cap, D], f32, kind="ExternalOutput")

    n_st = (cap + P - 1) // P          # 128-sample tiles for the combine
    SGS = 512                          # fp32 moving-operand free-dim limit
    sgroups = [(g, min(SGS, cap - g)) for g in range(0, cap, SGS)]

    with tile.TileContext(nc) as tc, ExitStack() as ctx:
        consts = ctx.enter_context(tc.tile_pool(name="consts", bufs=1))
        wa_pool = ctx.enter_context(tc.tile_pool(name="wa", bufs=4))
        psA = ctx.enter_context(tc.tile_pool(name="psA", bufs=4, space="PSUM"))
        psL = psA
        psC = ctx.enter_context(tc.tile_pool(name="psC", bufs=2, space="PSUM"))
        outp = ctx.enter_context(tc.tile_pool(name="outp", bufs=2))

        zT_sb = consts.tile([P, cap], f32r)
        nc.sync.dma_start(zT_sb[:], zT_d[:])
        W1z_sb = consts.tile([P, HID], f32r)
        nc.sync.dma_start(W1z_sb[:], W1z_d[:])
        b1c_sb = consts.tile([P, KC], f32)
        nc.sync.dma_start(b1c_sb[:], b1c_d[:])
        W2_sb = consts.tile([P, KC, HID], f32r)
        for k in range(KC):
            nc.sync.dma_start(W2_sb[:, k, :], W2r_d[:, k, :])
        b2r_sb = consts.tile([P, KC], f32)
        nc.sync.dma_start(b2r_sb[:], b2r_d[:])
        bac_sb = consts.tile([P, NCH], f32)
        nc.sync.dma_start(bac_sb[:], bac_d[:])
        x_all = consts.tile([P, NCH, DXT], f16)

        # ---- Phase A: cTMU (two gelu layers), activations as [hid, sample]
        h_sb = consts.tile([P, KC, cap], f32r)
        t_sb = consts.tile([P, KC, cap], f16)
        for (s0, slen) in sgroups:
            for j in range(KC):
                ph = psA.tile([P, slen], f32, tag="ps_mlp")
                nc.tensor.matmul(ph[:], W1z_sb[:, j * P:(j + 1) * P],
                                 zT_sb[:, s0:s0 + slen],
                                 start=True, stop=True)
                nc.scalar.activation(h_sb[:, j, s0:s0 + slen], ph[:], AF.Gelu,
                                     bias=b1c_sb[:, j:j + 1])
            for j in range(KC):
                pt = psA.tile([P, slen], f32, tag="ps_mlp")
                for k in range(KC):
                    nc.tensor.matmul(pt[:], W2_sb[:, k, j * P:(j + 1) * P],
                                     h_sb[:, k, s0:s0 + slen],
                                     start=(k == 0), stop=(k == KC - 1))
                nc.scalar.activation(t_sb[:, j, s0:s0 + slen], pt[:], AF.Gelu,
                                     bias=b2r_sb[:, j:j + 1])

        # ---- Phase B: routed IGU logits + fused mask/bias/exp
        e_all = consts.tile([P, NCH, cap], f16)
        for i in range(NCH):
            wa_t = wa_pool.tile([P, HID], f16, tag="wa")
            nc.sync.dma_start(wa_t[:], Wap_d[i])
            nc.sync.dma_start(x_all[:, i, :], Xp_d[i])
            for (s0, slen) in sgroups:
                pl = psL.tile([P, slen], f32, tag="ps_mlp")
                for k in range(KC):
                    nc.tensor.matmul(pl[:], wa_t[:, k * P:(k + 1) * P],
                                     t_sb[:, k, s0:s0 + slen],
                                     start=(k == 0), stop=(k == KC - 1))
                nc.scalar.activation(e_all[:, i, s0:s0 + slen], pl[:], AF.Exp,
                                     bias=bac_sb[:, i:i + 1])

        # ---- Phase C: convex combination; ones-column of Xp gives the denom
        for st in range(n_st):
            sz = min(P, cap - st * P)
            pa = psC.tile([P, 256], f32, tag="pa")
            pb = psC.tile([P, 258], f32, tag="pb")
            for i in range(NCH):
                lhs = e_all[:, i, st * P:st * P + sz]
                nc.tensor.matmul(pa[:sz, :], lhs,
                                 x_all[:, i, 0:256],
                                 start=(i == 0), stop=(i == NCH - 1))
                nc.tensor.matmul(pb[:sz, :], lhs,
                                 x_all[:, i, 256:514],
                                 start=(i == 0), stop=(i == NCH - 1))
            r = outp.tile([P, 1], f32, tag="recip")
            nc.vector.reciprocal(r[:sz], pb[:sz, 256:257])
            o = outp.tile([P, D], f32, tag="out")
            nc.vector.tensor_scalar_mul(o[:sz, 0:256], pa[:sz, :], r[:sz])
            nc.vector.tensor_scalar_mul(o[:sz, 256:512], pb[:sz, 0:256], r[:sz])
            nc.sync.dma_start(out_d[st * P:st * P + sz, :], o[:sz, :])

    nc.compile()
    return nc


def _get_compiled(cap: int):
    if cap not in _cache:
        _cache[cap] = _build(cap)
    return _cache[cap]


def kernel(z, class_ids, W1, b1, W2, b2, Wa, ba, Xbuf):
    from concourse.bass_utils import run_bass_kernel_spmd

    z = np.ascontiguousarray(np.asarray(z, np.float32))
    class_ids = np.asarray(class_ids).astype(np.int64)
    W1 = np.asarray(W1, np.float32)
    b1 = np.asarray(b1, np.float32)
    W2 = np.asarray(W2, np.float32)
    b2 = np.asarray(b2, np.float32)
    Wa = np.asarray(Wa, np.float32)
    ba = np.asarray(ba, np.float32)
    Xbuf = np.asarray(Xbuf, np.float32)

    B = z.shape[0]
    order = np.argsort(class_ids, kind="stable")
    counts = np.bincount(class_ids, minlength=C)
    cap = max(64, int(-(-counts.max() // 32) * 32))

    nc = _get_compiled(cap)

    W1z = np.ascontiguousarray(W1[:LATENT])
    W2r = np.ascontiguousarray(W2.reshape(KC, P, HID).transpose(1, 0, 2))
    b2r = np.ascontiguousarray(b2.reshape(KC, P).T)

    in_maps = []
    idx_by_class = []
    off = 0
    for c in range(C):
        n_c = int(counts[c])
        idx = order[off:off + n_c]
        off += n_c
        idx_by_class.append(idx)

        zTc = np.zeros((P, cap), np.float32)
        zTc[:, :n_c] = z[idx].T
        b1c = np.ascontiguousarray((b1 + W1[LATENT + c]).reshape(KC, P).T)
        Wap = np.ascontiguousarray(
            Wa[c].reshape(KC, P, NCH, P).transpose(2, 1, 0, 3).reshape(NCH, P, HID)
        ).astype(np.float16)
        bam = np.where(np.arange(NMAX) < COUNTS[c], ba[c], NEG).astype(np.float32)
        bac = np.ascontiguousarray(bam.reshape(NCH, P).T)
        Xp = np.zeros((NCH, P, DXT), np.float16)
        Xp[:, :, :D] = Xbuf[c].reshape(NCH, P, D)
        Xp[:, :, D] = 1.0

        in_maps.append({
            "zT": zTc, "W1z": W1z, "b1c": b1c, "W2r": W2r, "b2r": b2r,
            "Wap": Wap, "bac": bac, "Xp": np.ascontiguousarray(Xp),
        })

    trace = bool(os.environ.get("BASS_TRACE"))
    res = run_bass_kernel_spmd(
        nc, in_maps, core_ids=list(range(8)),
        trace=trace,
        trace_cores=list(range(8)) if trace else None,
    )
    global _last_results
    _last_results = res

    out = np.zeros((B, D), np.float32)
    for c in range(C):
        n_c = int(counts[c])
        if n_c:
            out[idx_by_class[c]] = res.results[c]["out"][:n_c]
    return out


_last_results = None


# revision 19
# speedup vs baseline: 1.1313x; 1.1051x over previous
"""Trainium2 Bass kernel for nn_ConvexGenerator (MoE-routed convex generator).

Expert-parallel sharding: core c owns class c's IGU weights (Wa[c], ba[c]) and
class buffer Xbuf[c]. Samples are routed by class_ids on the host (the
"all-to-all" is the host-side shard/unshard), so each core computes logits
only over its own class's columns -- the 8x headroom over the dense reference.

Per-core pipeline (all activations kept transposed, [feature, sample]):
  t  = gelu(gelu([z, onehot] @ W1 + b1) @ W2 + b2)         TensorE + ScalarE
  e  = exp(Wa_c.T @ t + ba_masked)  (bias folds mask+ba)   TensorE + ScalarE
  out= (e.T @ [X | 1]) ; num/den split off the ones column TensorE + VectorE
Softmax max-subtraction is skipped: logits are O(0.2) here, and masked columns
use a -1e9 bias so exp underflows to exactly 0.
"""

import os

import numpy as np

P = 128
B0 = 2048
LATENT = 128
C = 8
HID = 1024
D = 512
NMAX = 4096
COUNTS = np.array([1024, 1536, 2048, 2560, 3072, 3584, 3840, 4096])
NEG = -1e9
KC = HID // P     # 8 contraction chunks
NCH = NMAX // P   # 32 column chunks per class
DXT = D + 4       # X | ones | zero pad (even sizes for fp32r matmul)

_cache: dict = {}


def _build(cap: int):
    """Build + compile the per-core Tile program for sample capacity `cap`."""
    from contextlib import ExitStack

    import concourse.bacc as bacc
    import concourse.mybir as mybir
    import concourse.tile as tile

    f32 = mybir.dt.float32
    f32r = mybir.dt.float32r
    bf16 = mybir.dt.bfloat16
    f16 = mybir.dt.float16
    AF = mybir.ActivationFunctionType

    nc = bacc.Bacc("TRN2", target_bir_lowering=False, debug=False,
                   enable_asserts=False, num_devices=8)

    zT_d = nc.dram_tensor("zT", [P, cap], f16, kind="ExternalInput")
    W1z_d = nc.dram_tensor("W1z", [P, HID], f16, kind="ExternalInput")
    b1c_d = nc.dram_tensor("b1c", [P, KC], f32, kind="ExternalInput")
    W2r_d = nc.dram_tensor("W2r", [P, KC, HID], f16, kind="ExternalInput")
    b2r_d = nc.dram_tensor("b2r", [P, KC], f32, kind="ExternalInput")
    Wap_d = nc.dram_tensor("Wap", [NCH, P, HID], f16, kind="ExternalInput")
    bac_d = nc.dram_tensor("bac", [P, NCH], f32, kind="ExternalInput")
    Xp_d = nc.dram_tensor("Xp", [NCH, P, DXT], f16, kind="ExternalInput")
    out_d = nc.dram_tensor("out", [cap, D], f32, kind="ExternalOutput")

    n_st = (cap + P - 1) // P          # 128-sample tiles for the combine
    SGS = 512                          # fp32 moving-operand free-dim limit
    sgroups = [(g, min(SGS, cap - g)) for g in range(0, cap, SGS)]

    with tile.TileContext(nc) as tc, ExitStack() as ctx:
        consts = ctx.enter_context(tc.tile_pool(name="consts", bufs=1))
        wa_pool = ctx.enter_context(tc.tile_pool(name="wa", bufs=32))
        psA = ctx.enter_context(tc.tile_pool(name="psA", bufs=4, space="PSUM"))
        psL = psA
        psC = ctx.enter_context(tc.tile_pool(name="psC", bufs=2, space="PSUM"))
        outp = ctx.enter_context(tc.tile_pool(name="outp", bufs=2))

        zT_sb = consts.tile([P, cap], f16)
        nc.sync.dma_start(zT_sb[:], zT_d[:])
        W1z_sb = consts.tile([P, HID], f16)
        nc.sync.dma_start(W1z_sb[:], W1z_d[:])
        b1c_sb = consts.tile([P, KC], f32)
        nc.sync.dma_start(b1c_sb[:], b1c_d[:])
        W2_sb = consts.tile([P, KC, HID], f16)
        for k in range(KC):
            nc.sync.dma_start(W2_sb[:, k, :], W2r_d[:, k, :])
        b2r_sb = consts.tile([P, KC], f32)
        nc.sync.dma_start(b2r_sb[:], b2r_d[:])
        bac_sb = consts.tile([P, NCH], f32)
        nc.sync.dma_start(bac_sb[:], bac_d[:])
        x_all = consts.tile([P, NCH, DXT], f16)

        # ---- Phase A: cTMU (two gelu layers), activations as [hid, sample]
        h_sb = consts.tile([P, KC, cap], f16)
        t_sb = consts.tile([P, KC, cap], f16)
        for (s0, slen) in sgroups:
            for j in range(KC):
                ph = psA.tile([P, slen], f32, tag="ps_mlp")
                nc.tensor.matmul(ph[:], W1z_sb[:, j * P:(j + 1) * P],
                                 zT_sb[:, s0:s0 + slen],
                                 start=True, stop=True)
                nc.scalar.activation(h_sb[:, j, s0:s0 + slen], ph[:], AF.Gelu,
                                     bias=b1c_sb[:, j:j + 1])
            for j in range(KC):
                pt = psA.tile([P, slen], f32, tag="ps_mlp")
                for k in range(KC):
                    nc.tensor.matmul(pt[:], W2_sb[:, k, j * P:(j + 1) * P],
                                     h_sb[:, k, s0:s0 + slen],
                                     start=(k == 0), stop=(k == KC - 1))
                nc.scalar.activation(t_sb[:, j, s0:s0 + slen], pt[:], AF.Gelu,
                                     bias=b2r_sb[:, j:j + 1])

        # ---- Phase B: routed IGU logits + fused mask/bias/exp
        e_all = consts.tile([P, NCH, cap], f16)
        for i in range(NCH):
            wa_t = wa_pool.tile([P, HID], f16, tag="wa")
            nc.sync.dma_start(wa_t[:], Wap_d[i])
            nc.sync.dma_start(x_all[:, i, :], Xp_d[i])
            for (s0, slen) in sgroups:
                pl = psL.tile([P, slen], f32, tag="ps_mlp")
                for k in range(KC):
                    nc.tensor.matmul(pl[:], wa_t[:, k * P:(k + 1) * P],
                                     t_sb[:, k, s0:s0 + slen],
                                     start=(k == 0), stop=(k == KC - 1))
                nc.scalar.activation(e_all[:, i, s0:s0 + slen], pl[:], AF.Exp,
                                     bias=bac_sb[:, i:i + 1])

        # ---- Phase C: convex combination; ones-column of Xp gives the denom
        for st in range(n_st):
            sz = min(P, cap - st * P)
            pa = psC.tile([P, 256], f32, tag="pa")
            pb = psC.tile([P, 258], f32, tag="pb")
            for i in range(NCH):
                lhs = e_all[:, i, st * P:st * P + sz]
                nc.tensor.matmul(pa[:sz, :], lhs,
                                 x_all[:, i, 0:256],
                                 start=(i == 0), stop=(i == NCH - 1))
                nc.tensor.matmul(pb[:sz, :], lhs,
                                 x_all[:, i, 256:514],
                                 start=(i == 0), stop=(i == NCH - 1))
            r = outp.tile([P, 1], f32, tag="recip")
            nc.vector.reciprocal(r[:sz], pb[:sz, 256:257])
            o = outp.tile([P, D], f32, tag="out")
            nc.vector.tensor_scalar_mul(o[:sz, 0:256], pa[:sz, :], r[:sz])
            nc.vector.tensor_scalar_mul(o[:sz, 256:512], pb[:sz, 0:256], r[:sz])
            nc.sync.dma_start(out_d[st * P:st * P + sz, :], o[:sz, :])

    nc.compile()
    return nc


def _get_compiled(cap: int):
    if cap not in _cache:
        _cache[cap] = _build(cap)
    return _cache[cap]


def kernel(z, class_ids, W1, b1, W2, b2, Wa, ba, Xbuf):
    from concourse.bass_utils import run_bass_kernel_spmd

    z = np.ascontiguousarray(np.asarray(z, np.float32))
    class_ids = np.asarray(class_ids).astype(np.int64)
    W1 = np.asarray(W1, np.float32)
    b1 = np.asarray(b1, np.float32)
    W2 = np.asarray(W2, np.float32)
    b2 = np.asarray(b2, np.float32)
    Wa = np.asarray(Wa, np.float32)
    ba = np.asarray(ba, np.float32)
    Xbuf = np.asarray(Xbuf, np.float32)

    B = z.shape[0]
    order = np.argsort(class_ids, kind="stable")
    counts = np.bincount(class_ids, minlength=C)
    cap = max(64, int(-(-counts.max() // 32) * 32))

    nc = _get_compiled(cap)

    W1z = np.ascontiguousarray(W1[:LATENT]).astype(np.float16)
    W2r = np.ascontiguousarray(W2.reshape(KC, P, HID).transpose(1, 0, 2)).astype(np.float16)
    b2r = np.ascontiguousarray(b2.reshape(KC, P).T)

    in_maps = []
    idx_by_class = []
    off = 0
    for c in range(C):
        n_c = int(counts[c])
        idx = order[off:off + n_c]
        off += n_c
        idx_by_class.append(idx)

        zTc = np.zeros((P, cap), np.float16)
        zTc[:, :n_c] = z[idx].T.astype(np.float16)
        b1c = np.ascontiguousarray((b1 + W1[LATENT + c]).reshape(KC, P).T)
        Wap = np.ascontiguousarray(
            Wa[c].reshape(KC, P, NCH, P).transpose(2, 1, 0, 3).reshape(NCH, P, HID)
        ).astype(np.float16)
        bam = np.where(np.arange(NMAX) < COUNTS[c], ba[c], NEG).astype(np.float32)
        bac = np.ascontiguousarray(bam.reshape(NCH, P).T)
        Xp = np.zeros((NCH, P, DXT), np.float16)
        Xp[:, :, :D] = Xbuf[c].reshape(NCH, P, D)
        Xp[:, :, D] = 1.0

        in_maps.append({
            "zT": zTc, "W1z": W1z, "b1c": b1c, "W2r": W2r, "b2r": b2r,
            "Wap": Wap, "bac": bac, "Xp": np.ascontiguousarray(Xp),
        })

    trace = bool(os.environ.get("BASS_TRACE"))
    res = run_bass_kernel_spmd(
        nc, in_maps, core_ids=list(range(8)),
        trace=trace,
        trace_cores=list(range(8)) if trace else None,
    )
    global _last_results
    _last_results = res

    out = np.zeros((B, D), np.float32)
    for c in range(C):
        n_c = int(counts[c])
        if n_c:
            out[idx_by_class[c]] = res.results[c]["out"][:n_c]
    return out


_last_results = None
